# revision 2
# baseline (speedup 1.0000x reference)
"""Trainium2 Bass kernel for nn_BiAttention.

Data-parallel over batch across 8 NeuronCores (2 batches per core).

Per batch (QL=512, CL=2048, D=2048), with S[i,j] = sc[i] + sq[j] + G[i,j],
G = (c*wm) @ q^T:
  - alpha = softmax_j(S): the sc[i] term cancels in the row softmax, so we
    exponentiate E[j,i] = exp(G^T[j,i] + sq[j]) directly (values are O(1),
    no max subtraction needed) and normalize lazily via r[i] = 1/sum_j E.
  - context2question = (E^T @ q) * r  (normalization folded into the
    PSUM-evacuating multiply with c).
  - beta = softmax_i(max_j S) = softmax_i(sc[i] + log max_j E') ->
    b[i] = (max_j E[j,i]) * exp(sc[i]), q2c = (b @ c) / sum(b).

v2 vs baseline:
  - Device emits only the two computed output thirds (c*c2q, c*q2c) in
    bf16 ([B_loc, CL, 2D]); the host assembles the full fp32 output and
    fills the first third with the exact input `context`.  This cuts
    device HBM traffic from 136 MB to 72 MB per core (DMA was the top
    resource at 81% busy in the timeline model; bf16 rounding adds
    ~2e-3 rel error against a 2e-2 gate).
  - Output DMAs are one [128, 2048] store per row-block instead of 4
    [128,512] stores, and go out on the ACT/DVE DGE queues while loads
    stay on SP — fewer descriptors, no load/store head-of-line coupling.
  - A quarter of the O2 PSUM-evacuating multiplies run on the (idle)
    Pool engine instead of DVE.
  - q-phase tiles are parity-tagged by batch so batch b+1's q load/cast
    can overlap batch b's O3 tail.
"""

from contextlib import ExitStack

import numpy as np

import concourse.bass as bass
import concourse.mybir as mybir
import concourse.tile as tile
from concourse import bacc
from concourse.bass import ts
from concourse.bass_utils import run_bass_kernel_spmd
from concourse.masks import make_identity

P = 128
B, QL, CL, D = 16, 512, 2048, 2048
N_CORES = 8
B_LOC = B // N_CORES

F32 = mybir.dt.float32
BF16 = mybir.dt.bfloat16
AX = mybir.AxisListType
ALU = mybir.AluOpType
ACTF = mybir.ActivationFunctionType


CFG = {
    "io_bufs": 3,
    "outp_bufs": 4,
    "e_bufs": 2,
    "work_bufs": 2,
    "psb_bufs": 2,
    "ct_act_mod": 2,        # d % mod != 0 -> ACT, else DVE for cT evac
    "o2_pool_mod": 4,       # (ii*DB+db) % mod == 0 -> Pool else DVE
}


def emit(ctx, nc, tc, q_in, c_in, w_in, out, B_loc, QLd, CLd, Dd):
    """Emit the Tile program. Dimensions parameterized for small-shape sim."""
    JC = QLd // P          # j-chunks (4)
    DC = Dd // P           # d-chunks (16)
    IC = CLd // P          # i-chunks (16)
    IBW = min(512, CLd)    # i-block width
    IB = IBW // P          # i-chunks per block (4)
    NBLK = IC // IB        # i-blocks per batch (4)
    DBW = min(512, Dd)     # d-block width for M2
    DB = Dd // DBW         # d-blocks (4)

    const = ctx.enter_context(tc.tile_pool(name="const", bufs=1))
    io = ctx.enter_context(tc.tile_pool(name="io", bufs=CFG["io_bufs"]))
    res = ctx.enter_context(tc.tile_pool(name="res", bufs=1))
    ctp = ctx.enter_context(tc.tile_pool(name="ctp", bufs=1))
    work = ctx.enter_context(tc.tile_pool(name="work", bufs=CFG["work_bufs"]))
    work1 = ctx.enter_context(tc.tile_pool(name="work1", bufs=1))
    outp = ctx.enter_context(tc.tile_pool(name="outp", bufs=CFG["outp_bufs"]))
    small = ctx.enter_context(tc.tile_pool(name="small", bufs=2))
    rows = ctx.enter_context(tc.tile_pool(name="rows", bufs=1))
    ps_big = ctx.enter_context(tc.tile_pool(name="psb", bufs=CFG["psb_bufs"], space="PSUM"))
    ps_min = ctx.enter_context(tc.tile_pool(name="psm", bufs=1, space="PSUM"))

    # ---- constants ----
    wcol = const.tile([P, 3 * DC], F32)      # (p, col) = wsim[col*128 + p]
    nc.sync.dma_start(wcol, w_in.rearrange("(o p) -> p o", p=P))
    wcol_b = const.tile([P, 3 * DC], BF16)
    nc.vector.tensor_copy(wcol_b, wcol)
    wqf = io.tile([P, Dd], F32, tag="fio", name="wqf")
    nc.sync.dma_start(wqf, w_in[None, 0:Dd].to_broadcast([P, Dd]))
    wq_bc = const.tile([P, Dd], BF16)        # wq broadcast to all partitions
    nc.vector.tensor_copy(wq_bc, wqf)
    ident = const.tile([P, P], BF16)
    make_identity(nc, ident)
    ones_col = const.tile([P, 1], BF16)
    nc.vector.memset(ones_col, 1.0)
    ones_col_f = const.tile([P, 1], F32)
    nc.vector.memset(ones_col_f, 1.0)
    ones_11 = const.tile([1, 1], BF16)
    nc.vector.memset(ones_11, 1.0)

    def q_phase(b):
        # Q phase: load q, cast, sq, transposed+wm-scaled qmT
        pe = b % 2
        qb = []
        for j in range(JC):
            qf = io.tile([P, Dd], F32, tag="fio", name="qf")
            nc.sync.dma_start(qf, q_in[b, ts(j, P), :])
            t = res.tile([P, Dd], BF16, tag=f"qb{pe}_{j}", name=f"qb{pe}_{j}")
            nc.vector.tensor_copy(t, qf)
            qb.append(t)
        sq = []
        for j in range(JC):
            scr = work1.tile([P, Dd], BF16, tag="ttr_scr", name="ttr_scr")
            s = small.tile([P, 1], F32, tag=f"sq{pe}_{j}", name=f"sq{pe}_{j}")
            nc.vector.tensor_mul(scr, qb[j], wq_bc)
            nc.vector.reduce_sum(s, scr, axis=AX.X)
            sq.append(s)
        qmT = []
        for d in range(DC):
            tp = ps_big.tile([P, QLd], BF16, tag="tp", name="tp", bufs=CFG.get("tp_bufs", 2))
            for j in range(JC):
                nc.tensor.transpose(tp[:, ts(j, P)], qb[j][:, ts(d, P)], ident)
            t = res.tile([P, QLd], BF16, tag=f"qmT{pe}_{d}", name=f"qmT{pe}_{d}")
            # evacuate PSUM with per-partition (=per-d) wm scale folded in
            nc.scalar.activation(t, tp, ACTF.Copy, bias=0.0,
                                 scale=wcol[:, 2 * DC + d: 2 * DC + d + 1])
            qmT.append(t)
        return qb, sq, qmT

    for b in range(B_loc):
        qb, sq, qmT = q_phase(b)

        b_all = small.tile([P, IC], BF16, tag=f"ball{b % 2}", name="ball")
        q2c_acc = rows.tile([1, Dd], F32, tag=f"q2ca{b % 2}", name="q2ca")
        cb = [None] * IC

        for blk in range(NBLK):
            # ---- load c rows, bf16 cast ----
            for ii in range(IB):
                ig = blk * IB + ii
                cf = io.tile([P, Dd], F32, tag="fio", name="cf")
                nc.sync.dma_start(cf, c_in[b, ts(ig, P), :])
                t = res.tile([P, Dd], BF16, tag=f"cb{ig}", name=f"cb{ig}")
                nc.scalar.copy(t, cf)
                cb[ig] = t
            # ---- transpose block of c ----
            cT = []
            for d in range(DC):
                tp = ps_big.tile([P, IBW], BF16, tag="tp", name="tp", bufs=CFG.get("tp_bufs", 2))
                for ii in range(IB):
                    nc.tensor.transpose(tp[:, ts(ii, P)],
                                        cb[blk * IB + ii][:, ts(d, P)], ident)
                t = ctp.tile([P, IBW], BF16, tag=f"ct{d}", name=f"ct{d}",
                             bufs=CFG.get("ct_bufs", 1))
                if d % CFG["ct_act_mod"] == 0:
                    nc.vector.tensor_copy(t, tp)
                else:
                    nc.scalar.copy(t, tp)
                cT.append(t)
            # ---- M1: S^T = qm^T^T @ cT (contract d), then exp ----
            E = []
            for j in range(JC):
                m1 = ps_big.tile([P, IBW], F32, tag="m1", name="m1")
                for d in range(DC):
                    nc.tensor.matmul(m1, lhsT=qmT[d][:, ts(j, P)], rhs=cT[d],
                                     start=(d == 0), stop=(d == DC - 1))
                e = work.tile([P, IBW], BF16, tag=f"e{j}", name=f"e{j}",
                              bufs=CFG["e_bufs"])
                nc.scalar.activation(e, m1, ACTF.Exp, bias=sq[j], scale=1.0)
                E.append(e)
            # ---- Z row -> r = 1/Z, broadcast to all partitions ----
            z = ps_min.tile([1, IBW], F32, tag="mini", name="mini",
                            bufs=CFG.get("mini_bufs", 1))
            for j in range(JC):
                nc.tensor.matmul(z, lhsT=ones_col, rhs=E[j],
                                 start=(j == 0), stop=(j == JC - 1))
            r_row = small.tile([1, IBW], F32, tag="rrow", name="rrow")
            nc.vector.reciprocal(r_row, z)
            r_rowb = small.tile([1, IBW], BF16, tag="rrowb", name="rrowb")
            nc.vector.tensor_copy(r_rowb, r_row)
            r_bc = work.tile([P, IBW], BF16, tag="rbc", name="rbc")
            nc.gpsimd.partition_broadcast(r_bc, r_rowb)
            # ---- max over j (partition dim, unnormalized E) ----
            mx = work.tile([P, IBW], BF16, tag="mx", name="mx")
            nc.vector.tensor_copy(mx, E[0])
            for j in range(1, JC):
                nc.vector.tensor_max(mx, mx, E[j])
            # ---- normalize E in place: alpha^T = E * r (r per free-position) ----
            for j in range(JC):
                nc.vector.tensor_mul(E[j], E[j], r_bc)
            # ---- sc row for this block ----
            scp = ps_min.tile([1, IBW], F32, tag="mini", name="mini", bufs=CFG.get("mini_bufs", 1))
            for d in range(DC):
                nc.tensor.matmul(scp, lhsT=wcol_b[:, DC + d: DC + d + 1],
                                 rhs=cT[d], start=(d == 0), stop=(d == DC - 1))
            scrow = small.tile([1, IBW], BF16, tag="scrow", name="scrow")
            nc.scalar.copy(scrow, scp)
            for ii in range(IB):
                ig = blk * IB + ii
                mt = ps_min.tile([P, P], BF16, tag="minib", name="minib")
                nc.tensor.transpose(mt, mx[:, ts(ii, P)], ident)
                mcol = small.tile([P, 1], F32, tag="mcol", name="mcol")
                nc.vector.reduce_max(mcol, mt, axis=AX.X)
                scc = ps_min.tile([P, 1], F32, tag="mini", name="mini", bufs=CFG.get("mini_bufs", 1))
                nc.tensor.matmul(scc, lhsT=scrow[0:1, ts(ii, P)], rhs=ones_11)
                esc = small.tile([P, 1], F32, tag="esc", name="esc")
                nc.scalar.activation(esc, scc, ACTF.Exp)
                nc.vector.tensor_mul(b_all[:, ig:ig + 1], mcol, esc)
            # ---- q2c partial sums (contract i within block) ----
            for db in range(DB):
                qp = ps_min.tile([1, DBW], F32, tag="mini", name="mini", bufs=CFG.get("mini_bufs", 1))
                for ii in range(IB):
                    ig = blk * IB + ii
                    nc.tensor.matmul(qp, lhsT=b_all[:, ig:ig + 1],
                                     rhs=cb[ig][:, ts(db, DBW)],
                                     start=(ii == 0), stop=(ii == IB - 1))
                a_sl = q2c_acc[0:1, ts(db, DBW)]
                if blk == 0:
                    nc.vector.tensor_copy(a_sl, qp)
                else:
                    nc.vector.tensor_add(a_sl, qp, a_sl)

            # ---- M2 (contract j) + O2 = alpha^T.T @ q * c, fused evac ----
            for ii in range(IB):
                ig = blk * IB + ii
                o2 = outp.tile([P, Dd], BF16, tag="ob", name="o2")
                for db in range(DB):
                    u = ps_big.tile([P, DBW], F32, tag="m2", name="m2")
                    for j in range(JC):
                        nc.tensor.matmul(u, lhsT=E[j][:, ts(ii, P)],
                                         rhs=qb[j][:, ts(db, DBW)],
                                         start=(j == 0),
                                         stop=(j == JC - 1))
                    eng = (nc.gpsimd
                           if (ii * DB + db) % CFG["o2_pool_mod"] == 0
                           else nc.vector)
                    eng.tensor_mul(o2[:, ts(db, DBW)], u,
                                   cb[ig][:, ts(db, DBW)])
                nc.scalar.dma_start(out[b, ts(ig, P), 0:Dd], o2)

        # ---- finalize beta/q2c ----
        bs = small.tile([P, 1], F32, tag="bsum", name="bsum")
        nc.vector.reduce_sum(bs, b_all[:, 0:IC], axis=AX.X)
        zb = ps_min.tile([1, 1], F32, tag="mini", name="mini", bufs=CFG.get("mini_bufs", 1))
        nc.tensor.matmul(zb, lhsT=ones_col_f, rhs=bs)
        zbr = small.tile([1, 1], F32, tag="zbr", name="zbr")
        nc.vector.reciprocal(zbr, zb)
        q2cb_row = rows.tile([1, Dd], BF16, tag=f"q2cbr{b % 2}", name="q2cbr")
        nc.vector.tensor_scalar(q2cb_row, q2c_acc, zbr, None, ALU.mult)
        q2c_bc = res.tile([P, Dd], BF16, tag=f"q2cbc{b % 2}", name="q2cbc")
        nc.gpsimd.partition_broadcast(q2c_bc, q2cb_row)
        # ---- O3 = c * q2c (broadcast over rows) ----
        for ig in range(IC):
            o3 = outp.tile([P, Dd], BF16, tag="ob", name="o3")
            nc.vector.tensor_mul(o3, cb[ig], q2c_bc)
            nc.vector.dma_start(out[b, ts(ig, P), Dd:2 * Dd], o3)


def build(B_loc=B_LOC, QLd=QL, CLd=CL, Dd=D):
    nc = bacc.Bacc("TRN2", target_bir_lowering=False, debug=False,
                   enable_asserts=False, num_devices=1)
    q_in = nc.dram_tensor("question", [B_loc, QLd, Dd], F32,
                          kind="ExternalInput").ap()
    c_in = nc.dram_tensor("context", [B_loc, CLd, Dd], F32,
                          kind="ExternalInput").ap()
    w_in = nc.dram_tensor("wsim", [3 * Dd], F32, kind="ExternalInput").ap()
    out = nc.dram_tensor("out", [B_loc, CLd, 2 * Dd], BF16,
                         kind="ExternalOutput").ap()
    with tile.TileContext(nc) as tc, ExitStack() as ctx:
        emit(ctx, nc, tc, q_in, c_in, w_in, out, B_loc, QLd, CLd, Dd)
    nc.compile()
    return nc


_CACHED_NC = None


def _get_nc():
    global _CACHED_NC
    if _CACHED_NC is None:
        _CACHED_NC = build()
    return _CACHED_NC


def _shard(question, context, wsim):
    in_maps = []
    for i in range(N_CORES):
        in_maps.append({
            "question": np.ascontiguousarray(question[i * B_LOC:(i + 1) * B_LOC],
                                             dtype=np.float32),
            "context": np.ascontiguousarray(context[i * B_LOC:(i + 1) * B_LOC],
                                            dtype=np.float32),
            "wsim": np.ascontiguousarray(wsim, dtype=np.float32),
        })
    return in_maps


def kernel_raw(question, context, wsim, **run_kwargs):
    """Run and return the full BassKernelResults (for profiling)."""
    nc = _get_nc()
    in_maps = _shard(np.asarray(question), np.asarray(context),
                     np.asarray(wsim))
    res = run_bass_kernel_spmd(nc, in_maps, core_ids=list(range(N_CORES)),
                               **run_kwargs)
    return res


def kernel(question, context, wsim):
    res = kernel_raw(question, context, wsim)
    dev = np.concatenate([np.asarray(res.results[i]["out"])
                          for i in range(N_CORES)], axis=0)  # (B, CL, 2D) bf16
    full = np.empty((B, CL, 3 * D), np.float32)
    full[:, :, :D] = np.asarray(context, np.float32)  # first third is exact
    full[:, :, D:] = dev.astype(np.float32)
    return full


# revision 17
# speedup vs baseline: 1.5748x; 1.5748x over previous
"""Trainium2 Bass kernel for nn_BiAttention.

Data-parallel over batch across 8 NeuronCores (2 batches per core).

Per batch (QL=512, CL=2048, D=2048), with S[i,j] = sc[i] + sq[j] + G[i,j],
G = (c*wm) @ q^T:
  - alpha = softmax_j(S): the sc[i] term cancels in the row softmax, so we
    exponentiate E[j,i] = exp(G^T[j,i] + sq[j]) directly (values are O(1),
    no max subtraction needed) and normalize lazily via r[i] = 1/sum_j E.
  - context2question = (E^T @ q) * r  (normalization folded into the
    PSUM-evacuating multiply with c).
  - beta = softmax_i(max_j S) = softmax_i(sc[i] + log max_j E') ->
    b[i] = (max_j E[j,i]) * exp(sc[i]), q2c = (b @ c) / sum(b).

v2 vs baseline:
  - Device emits only the two computed output thirds (c*c2q, c*q2c) in
    bf16 ([B_loc, CL, 2D]); the host assembles the full fp32 output and
    fills the first third with the exact input `context`.  This cuts
    device HBM traffic from 136 MB to 72 MB per core (DMA was the top
    resource at 81% busy in the timeline model; bf16 rounding adds
    ~2e-3 rel error against a 2e-2 gate).
  - Output DMAs are one [128, 2048] store per row-block instead of 4
    [128,512] stores, and go out on the ACT/DVE DGE queues while loads
    stay on SP — fewer descriptors, no load/store head-of-line coupling.
  - A quarter of the O2 PSUM-evacuating multiplies run on the (idle)
    Pool engine instead of DVE.
  - q-phase tiles are parity-tagged by batch so batch b+1's q load/cast
    can overlap batch b's O3 tail.
"""

from contextlib import ExitStack

import numpy as np

import concourse.bass as bass
import concourse.mybir as mybir
import concourse.tile as tile
from concourse import bacc
from concourse.bass import ts
from concourse.bass_utils import run_bass_kernel_spmd
from concourse.masks import make_identity

P = 128
B, QL, CL, D = 16, 512, 2048, 2048
N_CORES = 8
B_LOC = B // N_CORES

F32 = mybir.dt.float32
BF16 = mybir.dt.bfloat16
AX = mybir.AxisListType
ALU = mybir.AluOpType
ACTF = mybir.ActivationFunctionType


CFG = {
    "io_bufs": 3,
    "outp_bufs": 3,
    "e_bufs": 2,
    "work_bufs": 2,
    "psb_bufs": 2,
    "ct_act_mod": 2,        # d % mod != 0 -> ACT, else DVE for cT evac
    "o3_pool_mod": 2,       # ig % mod != 0 -> Pool else DVE for O3 mul
    "st_eng": "act",        # output-store DGE queue: "act" or "sp"
    "qmt_evac_eng": "dve",  # qmT PSUM evac engine: "dve" or "act"
    "mini_bufs": 2,
}


def emit(ctx, nc, tc, q_in, c_in, w_in, out, B_loc, QLd, CLd, Dd):
    """Emit the Tile program. Dimensions parameterized for small-shape sim."""
    JC = QLd // P          # j-chunks (4)
    DC = Dd // P           # d-chunks (16)
    IC = CLd // P          # i-chunks (16)
    IBW = min(512, CLd)    # i-block width
    IB = IBW // P          # i-chunks per block (4)
    NBLK = IC // IB        # i-blocks per batch (4)
    DBW = min(512, Dd)     # d-block width for M2
    DB = Dd // DBW         # d-blocks (4)

    const = ctx.enter_context(tc.tile_pool(name="const", bufs=1))
    io = ctx.enter_context(tc.tile_pool(name="io", bufs=CFG["io_bufs"]))
    res = ctx.enter_context(tc.tile_pool(name="res", bufs=1))
    ctp = ctx.enter_context(tc.tile_pool(name="ctp", bufs=1))
    work = ctx.enter_context(tc.tile_pool(name="work", bufs=CFG["work_bufs"]))
    work1 = ctx.enter_context(tc.tile_pool(name="work1", bufs=1))
    outp = ctx.enter_context(tc.tile_pool(name="outp", bufs=CFG["outp_bufs"]))
    small = ctx.enter_context(tc.tile_pool(name="small", bufs=2))
    rows = ctx.enter_context(tc.tile_pool(name="rows", bufs=1))
    ps_big = ctx.enter_context(tc.tile_pool(name="psb", bufs=CFG["psb_bufs"], space="PSUM"))
    ps_min = ctx.enter_context(tc.tile_pool(name="psm", bufs=1, space="PSUM"))

    # ---- constants ----
    wcol = const.tile([P, 3 * DC], F32)      # (p, col) = wsim[col*128 + p]
    nc.sync.dma_start(wcol, w_in.rearrange("(o p) -> p o", p=P))
    wcol_b = const.tile([P, 3 * DC], BF16)
    nc.vector.tensor_copy(wcol_b, wcol)
    wqf = io.tile([P, Dd], F32, tag="fio", name="wqf")
    nc.sync.dma_start(wqf, w_in[None, 0:Dd].to_broadcast([P, Dd]))
    wq_bc = const.tile([P, Dd], BF16)        # wq broadcast to all partitions
    nc.vector.tensor_copy(wq_bc, wqf)
    ident = const.tile([P, P], BF16)
    make_identity(nc, ident)
    ones_col = const.tile([P, 1], BF16)
    nc.vector.memset(ones_col, 1.0)
    ones_col_f = const.tile([P, 1], F32)
    nc.vector.memset(ones_col_f, 1.0)
    ones_11 = const.tile([1, 1], BF16)
    nc.vector.memset(ones_11, 1.0)

    def q_phase(b):
        # Q phase: load q, cast, sq, transposed+wm-scaled qmT
        pe = b % 2
        qb = []
        for j in range(JC):
            qf = io.tile([P, Dd], F32, tag="fio", name="qf")
            nc.sync.dma_start(qf, q_in[b, ts(j, P), :])
            t = res.tile([P, Dd], BF16, tag=f"qb{pe}_{j}", name=f"qb{pe}_{j}")
            nc.vector.tensor_copy(t, qf)
            qb.append(t)
        sq = []
        for j in range(JC):
            scr = work1.tile([P, Dd], BF16, tag="ttr_scr", name="ttr_scr")
            s = small.tile([P, 1], F32, tag=f"sq{pe}_{j}", name=f"sq{pe}_{j}")
            nc.vector.tensor_mul(scr, qb[j], wq_bc)
            nc.vector.reduce_sum(s, scr, axis=AX.X)
            sq.append(s)
        qmT = []
        for d in range(DC):
            tp = ps_big.tile([P, QLd], BF16, tag="tp", name="tp", bufs=CFG.get("tp_bufs", 2))
            for j in range(JC):
                nc.tensor.transpose(tp[:, ts(j, P)], qb[j][:, ts(d, P)], ident)
            t = res.tile([P, QLd], BF16, tag=f"qmT{d}", name=f"qmT{d}")
            # evacuate PSUM with per-partition (=per-d) wm scale folded in
            if CFG["qmt_evac_eng"] == "dve":
                nc.vector.tensor_scalar(
                    t, tp, wcol[:, 2 * DC + d: 2 * DC + d + 1], None, ALU.mult)
            else:
                nc.scalar.activation(t, tp, ACTF.Copy, bias=0.0,
                                     scale=wcol[:, 2 * DC + d: 2 * DC + d + 1])
            qmT.append(t)
        return qb, sq, qmT

    for b in range(B_loc):
        qb, sq, qmT = q_phase(b)

        b_all = small.tile([P, IC], BF16, tag=f"ball{b % 2}", name="ball")
        q2c_acc = rows.tile([1, Dd], F32, tag="q2ca", name="q2ca")
        cb = [None] * IC

        for blk in range(NBLK):
            # ---- load c rows, bf16 cast ----
            for ii in range(IB):
                ig = blk * IB + ii
                cf = io.tile([P, Dd], F32, tag="fio", name="cf")
                nc.sync.dma_start(cf, c_in[b, ts(ig, P), :])
                t = res.tile([P, Dd], BF16, tag=f"cb{ig}", name=f"cb{ig}")
                nc.scalar.copy(t, cf)
                cb[ig] = t
            # ---- transpose block of c ----
            cT = []
            for d in range(DC):
                tp = ps_big.tile([P, IBW], BF16, tag="tp", name="tp", bufs=CFG.get("tp_bufs", 2))
                for ii in range(IB):
                    nc.tensor.transpose(tp[:, ts(ii, P)],
                                        cb[blk * IB + ii][:, ts(d, P)], ident)
                t = ctp.tile([P, IBW], BF16, tag=f"ct{d}", name=f"ct{d}",
                             bufs=CFG.get("ct_bufs", 1))
                if d % CFG["ct_act_mod"] == 0:
                    nc.vector.tensor_copy(t, tp)
                else:
                    nc.scalar.copy(t, tp)
                cT.append(t)
            # ---- M1: S^T = qm^T^T @ cT (contract d), then exp ----
            E = []
            for j in range(JC):
                m1 = ps_big.tile([P, IBW], F32, tag="m1", name="m1")
                for d in range(DC):
                    nc.tensor.matmul(m1, lhsT=qmT[d][:, ts(j, P)], rhs=cT[d],
                                     start=(d == 0), stop=(d == DC - 1))
                e = work.tile([P, IBW], BF16, tag=f"e{j}", name=f"e{j}",
                              bufs=CFG["e_bufs"])
                nc.scalar.activation(e, m1, ACTF.Exp, bias=sq[j], scale=1.0)
                E.append(e)
            # ---- Z row -> r = 1/Z, broadcast to all partitions ----
            z = ps_min.tile([1, IBW], F32, tag="mini", name="mini",
                            bufs=CFG.get("mini_bufs", 1))
            for j in range(JC):
                nc.tensor.matmul(z, lhsT=ones_col, rhs=E[j],
                                 start=(j == 0), stop=(j == JC - 1))
            r_row = small.tile([1, IBW], F32, tag="rrow", name="rrow", bufs=1)
            nc.vector.reciprocal(r_row, z)
            r_rowb = small.tile([1, IBW], BF16, tag="rrowb", name="rrowb", bufs=1)
            nc.vector.tensor_copy(r_rowb, r_row)
            r_bc = work.tile([P, IBW], BF16, tag="rbc", name="rbc")
            nc.gpsimd.partition_broadcast(r_bc, r_rowb)
            # ---- max over j (partition dim, unnormalized E) ----
            mx = work.tile([P, IBW], BF16, tag="mx", name="mx")
            nc.vector.tensor_copy(mx, E[0])
            for j in range(1, JC):
                nc.vector.tensor_max(mx, mx, E[j])
            # ---- normalize E in place: alpha^T = E * r (r per free-position) ----
            for j in range(JC):
                nc.vector.tensor_mul(E[j], E[j], r_bc)
            # ---- sc row for this block ----
            scp = ps_min.tile([1, IBW], F32, tag="mini", name="mini", bufs=CFG.get("mini_bufs", 1))
            for d in range(DC):
                nc.tensor.matmul(scp, lhsT=wcol_b[:, DC + d: DC + d + 1],
                                 rhs=cT[d], start=(d == 0), stop=(d == DC - 1))
            scrow = small.tile([1, IBW], BF16, tag="scrow", name="scrow", bufs=1)
            nc.scalar.copy(scrow, scp)
            for ii in range(IB):
                ig = blk * IB + ii
                mt = ps_min.tile([P, P], BF16, tag="minib", name="minib")
                nc.tensor.transpose(mt, mx[:, ts(ii, P)], ident)
                mcol = small.tile([P, 1], F32, tag="mcol", name="mcol")
                nc.vector.reduce_max(mcol, mt, axis=AX.X)
                scc = ps_min.tile([P, 1], F32, tag="mini", name="mini", bufs=CFG.get("mini_bufs", 1))
                nc.tensor.matmul(scc, lhsT=scrow[0:1, ts(ii, P)], rhs=ones_11)
                esc = small.tile([P, 1], F32, tag="esc", name="esc")
                nc.scalar.activation(esc, scc, ACTF.Exp)
                nc.vector.tensor_mul(b_all[:, ig:ig + 1], mcol, esc)
            # ---- q2c partial sums (contract i within block) ----
            for db in range(DB):
                qp = ps_min.tile([1, DBW], F32, tag="mini", name="mini", bufs=CFG.get("mini_bufs", 1))
                for ii in range(IB):
                    ig = blk * IB + ii
                    nc.tensor.matmul(qp, lhsT=b_all[:, ig:ig + 1],
                                     rhs=cb[ig][:, ts(db, DBW)],
                                     start=(ii == 0), stop=(ii == IB - 1))
                a_sl = q2c_acc[0:1, ts(db, DBW)]
                if blk == 0:
                    nc.vector.tensor_copy(a_sl, qp)
                else:
                    nc.vector.tensor_add(a_sl, qp, a_sl)

            # ---- M2 (contract j) + O2 = alpha^T.T @ q * c, fused evac ----
            for ii in range(IB):
                ig = blk * IB + ii
                o2 = outp.tile([P, Dd], BF16, tag="ob", name="o2")
                for db in range(DB):
                    u = ps_big.tile([P, DBW], F32, tag="m2", name="m2")
                    for j in range(JC):
                        nc.tensor.matmul(u, lhsT=E[j][:, ts(ii, P)],
                                         rhs=qb[j][:, ts(db, DBW)],
                                         start=(j == 0),
                                         stop=(j == JC - 1))
                    # Pool/GPSIMD cannot read PSUM -> O2 evac stays on DVE
                    nc.vector.tensor_mul(o2[:, ts(db, DBW)], u,
                                         cb[ig][:, ts(db, DBW)])
                st = nc.scalar if CFG["st_eng"] == "act" else nc.sync
                st.dma_start(out[b, ts(ig, P), 0:Dd], o2)

        # ---- finalize beta/q2c ----
        bs = small.tile([P, 1], F32, tag="bsum", name="bsum")
        nc.vector.reduce_sum(bs, b_all[:, 0:IC], axis=AX.X)
        zb = ps_min.tile([1, 1], F32, tag="mini", name="mini", bufs=CFG.get("mini_bufs", 1))
        nc.tensor.matmul(zb, lhsT=ones_col_f, rhs=bs)
        zbr = small.tile([1, 1], F32, tag="zbr", name="zbr")
        nc.vector.reciprocal(zbr, zb)
        q2cb_row = rows.tile([1, Dd], BF16, tag="q2cbr", name="q2cbr")
        nc.vector.tensor_scalar(q2cb_row, q2c_acc, zbr, None, ALU.mult)
        q2c_bc = res.tile([P, Dd], BF16, tag="q2cbc", name="q2cbc")
        nc.gpsimd.partition_broadcast(q2c_bc, q2cb_row)
        # ---- O3 = c * q2c (broadcast over rows) ----
        for ig in range(IC):
            o3 = outp.tile([P, Dd], BF16, tag="ob", name="o3")
            # SBUF-only multiply -> offload to the otherwise-idle Pool engine
            eng = nc.gpsimd if ig % CFG["o3_pool_mod"] != 0 else nc.vector
            eng.tensor_mul(o3, cb[ig], q2c_bc)
            st = nc.scalar if CFG["st_eng"] == "act" else nc.sync
            st.dma_start(out[b, ts(ig, P), Dd:2 * Dd], o3)


def build(B_loc=B_LOC, QLd=QL, CLd=CL, Dd=D):
    nc = bacc.Bacc("TRN2", target_bir_lowering=False, debug=False,
                   enable_asserts=False, num_devices=1)
    q_in = nc.dram_tensor("question", [B_loc, QLd, Dd], F32,
                          kind="ExternalInput").ap()
    c_in = nc.dram_tensor("context", [B_loc, CLd, Dd], F32,
                          kind="ExternalInput").ap()
    w_in = nc.dram_tensor("wsim", [3 * Dd], F32, kind="ExternalInput").ap()
    out = nc.dram_tensor("out", [B_loc, CLd, 2 * Dd], BF16,
                         kind="ExternalOutput").ap()
    with tile.TileContext(nc) as tc, ExitStack() as ctx:
        emit(ctx, nc, tc, q_in, c_in, w_in, out, B_loc, QLd, CLd, Dd)
    nc.compile()
    return nc


_CACHED_NC = None


def _get_nc():
    global _CACHED_NC
    if _CACHED_NC is None:
        _CACHED_NC = build()
    return _CACHED_NC


def _shard(question, context, wsim):
    in_maps = []
    for i in range(N_CORES):
        in_maps.append({
            "question": np.ascontiguousarray(question[i * B_LOC:(i + 1) * B_LOC],
                                             dtype=np.float32),
            "context": np.ascontiguousarray(context[i * B_LOC:(i + 1) * B_LOC],
                                            dtype=np.float32),
            "wsim": np.ascontiguousarray(wsim, dtype=np.float32),
        })
    return in_maps


def kernel_raw(question, context, wsim, **run_kwargs):
    """Run and return the full BassKernelResults (for profiling)."""
    nc = _get_nc()
    in_maps = _shard(np.asarray(question), np.asarray(context),
                     np.asarray(wsim))
    res = run_bass_kernel_spmd(nc, in_maps, core_ids=list(range(N_CORES)),
                               **run_kwargs)
    return res


def kernel(question, context, wsim):
    res = kernel_raw(question, context, wsim)
    dev = np.concatenate([np.asarray(res.results[i]["out"])
                          for i in range(N_CORES)], axis=0)  # (B, CL, 2D) bf16
    full = np.empty((B, CL, 3 * D), np.float32)
    full[:, :, :D] = np.asarray(context, np.float32)  # first third is exact
    full[:, :, D:] = dev.astype(np.float32)
    return full
